# revision 63
# baseline (speedup 1.0000x reference)
"""Trainium2 Bass kernel for BasicCNN_LSTM (3x conv3x3+relu -> BN -> GAP -> LSTM -> BN -> dense).

Sharding: data-parallel over batch across 8 NeuronCores (4 batches/core).

Per-core plan (128 frames = 4 batches x 32 timesteps, processed as 64 frame-pairs):
  - conv matmuls in bf16 (1 cyc/row vs 4 for fp32), fp32 PSUM accumulation.
  - conv1 (C=1 -> 48): host-built im2col block-diagonal K=18 matmuls.
  - conv2/conv3 (48 -> 48): 9 tap-accumulated matmuls, 4 concurrent PE
    quadrants; psum bank layout pairs one row-block of EACH image half per
    bank so evacuation is 1 merged [0:112] op + 2 crossed ops per stage.
  - halo rows copied SBUF->SBUF on GpSimd; A tiles persistent (pads zeroed once).
  - conv bias via activation bias; BN1 folded into LSTM input weights;
    GAP via ACT relu-in-place + DVE tensor_reduce; Z assembly on GpSimd.
  - LSTM: transposed gates -- Z [112,4] and h [8,4] are the stationary
    matmul operands (4-column LDWEIGHTS), gates land as psum columns
    [4 batch, 32 gates]; 2 column-sliced activations; h fed back via a
    32x32 DVE transpose.  BN2 + output dense as a DVE mul+reduce head.
  - emission order per iteration: evac(p-1), evac2(p-2), conv3(p-3), conv1(p),
    conv2(p-1), pool(p-4), lstm -- keeps the PE fed (no HAM oscillation).
"""

import sys

sys.path.insert(0, "/opt/trn_rl_repo")

import numpy as np
import ml_dtypes

_NCORES = 8
_B, _T, _HW, _F, _U = 32, 32, 28, 48, 8
_EPS = 1e-3
_BPC = _B // _NCORES          # batches per core (4)
_PAIRS = _BPC * _T // 2       # frame pairs per core (64)

_F32 = np.float32
_BF16 = ml_dtypes.bfloat16


# ---------------------------------------------------------------------------
# Device program
# ---------------------------------------------------------------------------

def _build_program():
    import concourse.bass as bass  # noqa: F401
    import concourse.tile as tile
    from concourse.tile import add_dep_helper
    from concourse import bacc, mybir

    f32 = mybir.dt.float32
    bf16 = mybir.dt.bfloat16
    AF = mybir.ActivationFunctionType
    ALU = mybir.AluOpType
    AX = mybir.AxisListType

    nc = bacc.Bacc("TRN2", target_bir_lowering=False, debug=False, num_devices=_NCORES)

    # DRAM I/O
    x1_d = nc.dram_tensor("x1", (_PAIRS, 2, 18, 2, 196), bf16, kind="ExternalInput")
    w1_d = nc.dram_tensor("w1t", (128, 128), bf16, kind="ExternalInput")
    w2_d = nc.dram_tensor("w2t", (128, 9, 64), bf16, kind="ExternalInput")
    w3_d = nc.dram_tensor("w3t", (128, 9, 64), bf16, kind="ExternalInput")
    b1_d = nc.dram_tensor("b1t", (128, 1), f32, kind="ExternalInput")
    b2_d = nc.dram_tensor("b2t", (128, 1), f32, kind="ExternalInput")
    b3_d = nc.dram_tensor("b3t", (128, 1), f32, kind="ExternalInput")
    wf_d = nc.dram_tensor("wfm", (128, 32), f32, kind="ExternalInput")
    wh_d = nc.dram_tensor("whm", (8, 32), f32, kind="ExternalInput")
    bo_d = nc.dram_tensor("bot4", (4, 1), f32, kind="ExternalInput")
    wo_d = nc.dram_tensor("wob", (4, 256), f32, kind="ExternalInput")
    out_d = nc.dram_tensor("out", (4, 32), f32, kind="ExternalOutput")

    from contextlib import ExitStack
    with tile.TileContext(nc) as tc, ExitStack() as ctx:
        # ---- persistent tiles (one const pool, unique tag per tile) ----
        cp = ctx.enter_context(tc.tile_pool(name="const", bufs=1))
        W1T = cp.tile([128, 128], bf16, name="W1T", tag="W1T")
        W2T = cp.tile([128, 9, 64], bf16, name="W2T", tag="W2T")
        W3T = cp.tile([128, 9, 64], bf16, name="W3T", tag="W3T")
        B1T = cp.tile([128, 1], f32, name="B1T", tag="B1T")
        B2T = cp.tile([128, 1], f32, name="B2T", tag="B2T")
        B3T = cp.tile([128, 1], f32, name="B3T", tag="B3T")
        WFM = cp.tile([128, 32], f32, name="WFM", tag="WFM")
        WHM = cp.tile([8, 32], f32, name="WHM", tag="WHM")
        BOT = cp.tile([4, 1], f32, name="BOT", tag="BOT")
        WOB = cp.tile([4, 256], f32, name="WOB", tag="WOB")
        HST = cp.tile([4, 256], f32, name="HST", tag="HST")   # [batch, 8t+u]
        ZT = [cp.tile([128, 4], f32, name=f"ZT{i}", tag=f"ZT{i}") for i in range(3)]
        CT = [cp.tile([4, 8], f32, name=f"CT{i}", tag=f"CT{i}") for i in range(2)]
        # h as [4 batch, 8 unit] in a 32x32 block + its DVE-transposed [8, 4]
        HN32 = [cp.tile([32, 32], f32, name=f"HN{i}", tag=f"HN{i}") for i in range(2)]
        HNT = [cp.tile([32, 32], f32, name=f"HNT{i}", tag=f"HNT{i}") for i in range(2)]
        PC = [cp.tile([128, 2], f32, name=f"PC{i}", tag=f"PC{i}") for i in range(4)]
        # persistent activation tiles, manually rotated (pads zeroed once)
        A1T = [cp.tile([128, 2, 16, 30], bf16, name=f"A1T{i}", tag=f"A1T{i}")
               for i in range(3)]
        A2T = [cp.tile([128, 2, 16, 30], bf16, name=f"A2T{i}", tag=f"A2T{i}")
               for i in range(3)]

        # warmup-critical first: A1T[0] zeroed early (gpsimd queue kept clear),
        # W1T on sync ahead of the x1 stream.
        nc.gpsimd.memset(A1T[0][:, :, :, :], 0.0)
        nc.sync.dma_start(W1T[:, :], w1_d.ap()[:, :])
        nc.scalar.dma_start(W2T[:, :, :], w2_d.ap()[:, :, :])
        nc.scalar.dma_start(W3T[:, :, :], w3_d.ap()[:, :, :])
        nc.sync.dma_start(B1T[:, :], b1_d.ap()[:, :])
        nc.scalar.dma_start(B2T[:, :], b2_d.ap()[:, :])
        nc.scalar.dma_start(B3T[:, :], b3_d.ap()[:, :])
        nc.scalar.dma_start(WFM[:, :], wf_d.ap()[:, :])
        nc.scalar.dma_start(WHM[:, :], wh_d.ap()[:, :])
        nc.scalar.dma_start(WOB[:, :], wo_d.ap()[:, :])
        nc.scalar.dma_start(BOT[:, :], bo_d.ap()[:, :])

        # Z rows 48:64 := 1.0 once (bias "ones" rows)
        for z in ZT:
            nc.vector.memset(z[32:64, :], 1.0)
        nc.vector.memset(CT[0][:, :], 0.0)
        for hh in HN32 + HNT:
            nc.vector.memset(hh[:, :], 0.0)
        # zero whole A tiles once: pads (rows 0/15, cols 0/29) stay zero forever
        for a in A1T[1:] + A2T:
            nc.gpsimd.memset(a[:, :, :, :], 0.0)



        # ---- pools ----
        x1_pool = ctx.enter_context(tc.tile_pool(name="x1p", bufs=6))
        p1_pool = ctx.enter_context(tc.tile_pool(name="p1p", bufs=3, space="PSUM"))
        p2_pool = ctx.enter_context(tc.tile_pool(name="p2p", bufs=2, space="PSUM"))
        p3_pool = ctx.enter_context(tc.tile_pool(name="p3p", bufs=2, space="PSUM"))
        g_pool = ctx.enter_context(tc.tile_pool(name="gp", bufs=1, space="PSUM"))
        ls_pool = ctx.enter_context(tc.tile_pool(name="lsp", bufs=4))

        # HAM warmup: back-to-back dummy matmuls during the DMA preamble
        # flip the PE clock gate to 8/8 before real conv work.
        for wu in range(20):
            WUf = p1_pool.tile([128, 512], f32, name="WU", tag="c1ps")
            nc.tensor.matmul(WUf[0:112, 0:392], lhsT=W1T[0:18, 0:112],
                             rhs=A1T[0][0:18, 0:1, 0:14, 0:28],
                             skip_group_check=True)

        # per-pair state carried between loop stages
        X1 = [None] * _PAIRS
        P1 = [None] * _PAIRS   # (PA, PB, mA_last, mB_last)
        P2 = [None] * _PAIRS
        P3 = [None] * _PAIRS
        GTS = [None] * _T

        def conv1(p):
            # block-diagonal K=18/M=112 packing. Slot 0 (X rows 0:18):
            # krows 0:9 = half0 out-rows 0-6 -> psum 0:48; krows 9:18 =
            # half1 out-rows 14-20 -> psum 64:112.  Slot 1 (X rows 64:82):
            # krows 0:9 = half1 rows 21-27; krows 9:18 = half0 rows 7-13.
            X = x1_pool.tile([128, 2, 196], bf16, name="X")
            nc.sync.dma_start(X[0:18, :, :], x1_d.ap()[p, 0])
            nc.sync.dma_start(X[64:82, :, :], x1_d.ap()[p, 1])
            X1[p] = X
            PAf = p1_pool.tile([128, 512], f32, name="PAf", tag="c1ps")
            PBf = p1_pool.tile([128, 512], f32, name="PBf", tag="c1ps")
            PA, PB = PAf[:, 0:392], PBf[:, 0:392]
            mA = nc.tensor.matmul(PA[0:112, :], lhsT=W1T[0:18, 0:112], rhs=X[0:18, :, :],
                                  skip_group_check=True)
            mB = nc.tensor.matmul(PB[0:112, :], lhsT=W1T[64:82, 0:112], rhs=X[64:82, :, :],
                                  skip_group_check=True)
            P1[p] = (PA, PB, mA, mB)

        def conv23(p, W, A, pool, tag, Pout):
            # psum bank layout: PA = [half0 rows 0-6 @ 0:48 | half1 rows 14-20
            # @ 64:112], PB = [half1 rows 21-27 @ 0:48 | half0 rows 7-13 @
            # 64:112].  4 disjoint PE quadrants per tap.
            PAf = pool.tile([128, 512], f32, name="PAf" + tag, tag=tag)
            PBf = pool.tile([128, 512], f32, name="PBf" + tag, tag=tag)
            PA, PB = PAf[:, 0:392], PBf[:, 0:392]
            # tap order dy=1, dy=2, dy=0: the dy=2 taps need halo H1 (ACT
            # e1 -> gpsimd) and dy=0 need H2 (DVE e3 -> gpsimd); running
            # dy=1 first lets the matmuls start before the halos land.
            for tap in (3, 4, 5, 6, 7, 8, 0, 1, 2):
                dy, dx = tap // 3, tap % 3
                st = tap == 3
                sp = tap == 2
                lo = W[0:48, tap, :]
                hi = W[64:112, tap, :]
                nc.tensor.matmul(PA[0:64, :], lhsT=lo,
                                 rhs=A[0:48, :, dy:dy + 7, dx:dx + 28],
                                 start=st, stop=sp, skip_group_check=True)
                mA = nc.tensor.matmul(PA[64:128, :], lhsT=hi,
                                 rhs=A[64:112, :, dy:dy + 7, dx:dx + 28],
                                 start=st, stop=sp, skip_group_check=True)
                nc.tensor.matmul(PB[64:128, :], lhsT=lo,
                                 rhs=A[0:48, :, 7 + dy:14 + dy, dx:dx + 28],
                                 start=st, stop=sp, skip_group_check=True)
                mB = nc.tensor.matmul(PB[0:64, :], lhsT=hi,
                                 rhs=A[64:112, :, 7 + dy:14 + dy, dx:dx + 28],
                                 start=st, stop=sp, skip_group_check=True)
            Pout[p] = (PA, PB, mA, mB)

        def relu_store(p, Psrc, B, A, act_merged):
            # psum -> padded split act tile, relu + bias.
            #  E1 (merged):  A[0:112, rows 1:8]  <- PA[0:112]   (same offset)
            #  E2 (crossed): A[64:112, rows 8:15] <- PB[0:48]   (half1 r21-27)
            #  E3 (crossed): A[0:48,  rows 8:15] <- PB[64:112]  (half0 r7-13)
            #  halo rows via GpSimd SBUF copies after E1/E3.
            PA, PB, mA, mB = Psrc[p]
            if act_merged:
                # stage 1: consumer (conv2) is same-iteration -- split for
                # minimum latency: big merged op on ACT, crossed pair on DVE.
                e1 = nc.scalar.activation(A[0:112, :, 1:8, 1:29], PA[0:112, :],
                                          AF.Relu, bias=B[0:112, :])
                e2 = nc.vector.tensor_scalar(A[64:112, :, 8:15, 1:29], PB[0:48, :],
                                             B[0:48, :], 0.0, ALU.add, ALU.max)
                e3 = nc.vector.tensor_scalar(A[0:48, :, 8:15, 1:29], PB[64:112, :],
                                             B[64:112, :], 0.0, ALU.add, ALU.max)
            else:
                # stage 2: consumer (conv3) is next iteration -- crossed pair
                # on ACT, merged op on DVE (balances against the DVE pool
                # reduces while keeping R1's ops first in both queues).
                e1 = nc.vector.tensor_scalar(A[0:112, :, 1:8, 1:29], PA[0:112, :],
                                             B[0:112, :], 0.0, ALU.add, ALU.max)
                e2 = nc.scalar.activation(A[64:112, :, 8:15, 1:29], PB[0:48, :],
                                          AF.Relu, bias=B[0:48, :])
                e3 = nc.scalar.activation(A[0:48, :, 8:15, 1:29], PB[64:112, :],
                                          AF.Relu, bias=B[64:112, :])
            # PE-W + engine-R same-bank hazard: order readers after the bank's
            # last matmul (PE completes in program order).
            add_dep_helper(e1.ins, mA.ins, reason="psum bank PA fully written")
            for rd in (e2, e3):
                add_dep_helper(rd.ins, mB.ins, reason="psum bank PB fully written")
            # halo: half0 row 15 = img row 14 = half1 storage row 1 (E1);
            #       half1 row 0 = img row 13 = half0 storage row 14 (E3).
            nc.gpsimd.tensor_copy(A[0:48, :, 15:16, 1:29], A[64:112, :, 1:2, 1:29])
            nc.gpsimd.tensor_copy(A[64:112, :, 0:1, 1:29], A[0:48, :, 14:15, 1:29])

        def pool3(p):
            # conv3 psum -> relu in place (ACT) -> per-frame sums (DVE
            # tensor_reduce; avoids the per-op ACT ACC_READ cost) -> Z
            # columns (GpSimd adds).
            PA, PB, mA, mB = P3[p]
            t, j = p // 2, p % 2
            Z = ZT[t % 3]
            ra = nc.scalar.activation(PA[0:112, :], PA[0:112, :],
                                      AF.Relu, bias=B3T[0:112, :])
            rb = nc.scalar.activation(PB[0:112, :], PB[0:112, :],
                                      AF.Relu, bias=B3T[0:112, :])
            add_dep_helper(ra.ins, mA.ins, reason="psum bank PA fully written")
            add_dep_helper(rb.ins, mB.ins, reason="psum bank PB fully written")
            pcA = PC[2 * (p % 2)]
            pcB = PC[2 * (p % 2) + 1]
            pa3 = PA.rearrange("p (f c) -> p f c", f=2)
            pb3 = PB.rearrange("p (f c) -> p f c", f=2)
            nc.vector.tensor_reduce(pcA[0:112, :], pa3[0:112, :, :], AX.X, ALU.add)
            nc.vector.tensor_reduce(pcB[0:112, :], pb3[0:112, :, :], AX.X, ALU.add)
            # Z col = 2*j + fi; both partition ranges summed by lstm matmul
            nc.gpsimd.tensor_add(Z[0:48, 2 * j:2 * j + 2], pcA[0:48, :], pcB[0:48, :])
            nc.gpsimd.tensor_add(Z[64:112, 2 * j:2 * j + 2], pcA[64:112, :], pcB[64:112, :])

        def lstm_mm(t):
            # transposed gates: G[batch 0:4, gate-cols f 0:8 | i 8:16 | o
            # 16:24 | g 24:32].  Z and h are the (tiny) stationary operands.
            # Emitted at the START of the iteration after Z is assembled so
            # the PE never waits on it (no FIFO head-of-line blocking).
            Z = ZT[t % 3]
            Gf = g_pool.tile([128, 512], f32, name="Gf", tag="gps")
            G = Gf[0:4, 0:32]
            nc.tensor.matmul(G[:, :], lhsT=Z[0:112, :], rhs=WFM[0:112, :],
                             start=True, stop=False)
            nc.tensor.matmul(G[:, :], lhsT=HNT[t % 2][0:8, 0:4], rhs=WHM[:, :],
                             start=False, stop=True)
            GTS[t] = G

        def lstm_rest(t):
            G = GTS[t]
            S = ls_pool.tile([4, 32], f32, name="S")
            nc.scalar.activation(S[:, 0:24], G[:, 0:24], AF.Sigmoid)
            nc.scalar.activation(S[:, 24:32], G[:, 24:32], AF.Tanh)
            Cp, Cn = CT[t % 2], CT[(t + 1) % 2]
            T1 = ls_pool.tile([4, 8], f32, name="T1")
            T2 = ls_pool.tile([4, 8], f32, name="T2")
            nc.gpsimd.tensor_mul(T1[:, :], S[:, 0:8], Cp[:, :])
            nc.gpsimd.tensor_mul(T2[:, :], S[:, 8:16], S[:, 24:32])
            nc.gpsimd.tensor_add(Cn[:, :], T1[:, :], T2[:, :])
            TC = ls_pool.tile([4, 8], f32, name="TC")
            nc.scalar.activation(TC[:, :], Cn[:, :], AF.Tanh)
            HN = HN32[(t + 1) % 2]
            nc.gpsimd.tensor_mul(HN[0:4, 0:8], S[:, 16:24], TC[:, :])
            nc.vector.transpose(HNT[(t + 1) % 2][:, :], HN[:, :])
            nc.gpsimd.tensor_copy(HST[0:4, 8 * t:8 * t + 8], HN[0:4, 0:8])

        # ---- software-pipelined emission (PE-ready work first) ----
        # lstm gate matmuls run at the start of the iteration AFTER their Z
        # was assembled; pools trail conv3 by one iteration in steady state
        # and chase it within the iteration during the drain.
        next_pool = 0
        next_mm = 0
        next_rest = 0
        for p in range(_PAIRS + 3):
            while next_mm < _T and 2 * next_mm + 1 < next_pool:
                lstm_mm(next_mm)
                next_mm += 1
            if 1 <= p < _PAIRS + 1:
                q = p - 1
                relu_store(q, P1, B1T, A1T[q % 3], act_merged=True)
            if 2 <= p < _PAIRS + 2:
                q = p - 2
                relu_store(q, P2, B2T, A2T[q % 3], act_merged=False)
            if 3 <= p < _PAIRS + 3:
                q = p - 3
                conv23(q, W3T, A2T[q % 3], p3_pool, "c3ps", P3)
            if p < _PAIRS:
                conv1(p)
            if 1 <= p < _PAIRS + 1:
                q = p - 1
                conv23(q, W2T, A1T[q % 3], p2_pool, "c2ps", P2)
            pool_limit = (p - 4) if p < _PAIRS else (p - 3)
            while next_pool <= pool_limit and next_pool < _PAIRS:
                pool3(next_pool)
                next_pool += 1
            if p >= _PAIRS:
                # drain: rest(t) must precede mm(t+1) (shared gates bank)
                while next_rest < next_mm:
                    lstm_rest(next_rest)
                    next_rest += 1
                while next_mm < _T and 2 * next_mm + 1 < next_pool:
                    lstm_mm(next_mm)
                    next_mm += 1
                for _ in range(4):
                    WUf = p1_pool.tile([128, 512], f32, name="WU2", tag="c1ps")
                    nc.tensor.matmul(WUf[0:112, 0:392], lhsT=W1T[0:18, 0:112],
                                     rhs=A1T[0][0:18, 0:1, 0:14, 0:28],
                                     skip_group_check=True)
            while next_rest < next_mm:
                lstm_rest(next_rest)
                next_rest += 1

        # ---- output head: y[b, t] = sum_u HST[b, 8t+u] * wob[u] + bot ----
        TP = cp.tile([4, 256], f32, name="TP", tag="TP")
        OUTS = cp.tile([4, 32], f32, name="OUTS", tag="OUTS")
        nc.vector.tensor_mul(TP[:, :], HST[:, :], WOB[:, :])
        tp3 = TP.rearrange("p (t u) -> p t u", u=8)
        YV = cp.tile([4, 32], f32, name="YV", tag="YV")
        nc.vector.tensor_reduce(YV[:, :], tp3[:, :, :], AX.X, ALU.add)
        nc.vector.tensor_scalar(OUTS[:, :], YV[:, :], BOT[0:4, :], None, ALU.add)
        nc.sync.dma_start(out_d.ap()[:, :], OUTS[:, :])

    nc.compile()
    return nc


# ---------------------------------------------------------------------------
# Host-side prep
# ---------------------------------------------------------------------------

def _prep_core_inputs(xc, w1, b1, w2, b2, w3, b3, bn1, wf, bf, wi1, bi1, wi2, bi2,
                      wo, bo, bn2, w_out, b_out):
    """xc: [4, 32, 28, 28, 1] float32 for one core. Returns the in_map dict."""
    T, HW = _T, _HW
    xp = np.zeros((_BPC, T, 30, 30), _F32)
    xp[:, :, 1:29, 1:29] = xc[..., 0]

    # im2col for conv1, block-diagonal packed to match the psum layout:
    # slot 0: krows 0:9 = half0 out-rows 0-6, krows 9:18 = half1 rows 14-20
    # slot 1: krows 0:9 = half1 rows 21-27,   krows 9:18 = half0 rows 7-13
    X1 = np.empty((_PAIRS, 2, 18, 2, 196), _F32)
    X1v = X1.reshape(T, 2, 2, 18, 2, 196)  # [t, j, slot, krow, fi, n]
    for h in range(2):
        for dy in range(3):
            for dx in range(3):
                blk = xp[:, :, h * 14 + dy:h * 14 + dy + 14, dx:dx + 28]  # [b, t, 14, 28]
                blk0 = blk[:, :, 0:7].reshape(_BPC, T, 196)
                blk1 = blk[:, :, 7:14].reshape(_BPC, T, 196)
                for j in range(2):
                    for fi in range(2):
                        if h == 0:
                            X1v[:, j, 0, 3 * dy + dx, fi] = blk0[2 * j + fi]
                            X1v[:, j, 1, 9 + 3 * dy + dx, fi] = blk1[2 * j + fi]
                        else:
                            X1v[:, j, 0, 9 + 3 * dy + dx, fi] = blk0[2 * j + fi]
                            X1v[:, j, 1, 3 * dy + dx, fi] = blk1[2 * j + fi]

    def wpack1():
        # block diagonal: K rows 0:9 -> out cols 0:48, rows 9:18 -> cols 64:112
        w = np.zeros((128, 128), _F32)
        w9 = w1.reshape(9, _F)
        w[0:9, 0:48] = w9
        w[9:18, 64:112] = w9
        w[64:73, 0:48] = w9
        w[73:82, 64:112] = w9
        return w

    def wpack(wn):
        w = np.zeros((128, 9, 64), _F32)
        for tap in range(9):
            m = wn[tap // 3, tap % 3]  # [48, 48]
            w[0:48, tap, 0:48] = m
            w[64:112, tap, 0:48] = m
        return w

    def bpack(bn):
        b = np.zeros((128, 1), _F32)
        b[0:48, 0] = bn
        b[64:112, 0] = bn
        return b

    bn1_g, bn1_b, bn1_m, bn1_v = bn1
    bn2_g, bn2_b, bn2_m, bn2_v = bn2
    s1 = bn1_g / np.sqrt(bn1_v + _EPS)
    t1 = bn1_b - bn1_m * s1
    Wx = np.concatenate([wf[:_F], wi1[:_F], wo[:_F], wi2[:_F]], axis=1)  # [48, 32] f,i,o,g
    Wh = np.concatenate([wf[_F:], wi1[_F:], wo[_F:], wi2[_F:]], axis=1)  # [8, 32]
    bias = np.concatenate([bf, bi1, bo, bi2]) + t1 @ Wx                  # [32]
    Wxs = (s1[:, None] * Wx) / float(HW * HW)

    # transposed gate weights: cols = f 0:8 | i 8:16 | o 16:24 | g 24:32
    WF = np.zeros((128, 32), _F32)
    WF[0:48] = Wxs
    WF[64:112] = Wxs
    WF[48] = bias                          # Z rows 48:64 are ones; bias on row 48
    WH = np.ascontiguousarray(Wh, _F32)    # [8, 32]

    s2 = bn2_g / np.sqrt(bn2_v + _EPS)
    t2 = bn2_b - bn2_m * s2
    wob = np.tile(s2 * w_out[:, 0], _T)[None, :].repeat(4, axis=0)  # [4, 256]
    bot4 = np.full((4, 1), t2 @ w_out[:, 0] + b_out[0], _F32)

    return {
        "x1": X1.astype(_BF16),
        "w1t": wpack1().astype(_BF16),
        "w2t": wpack(w2).astype(_BF16), "w3t": wpack(w3).astype(_BF16),
        "b1t": bpack(b1), "b2t": bpack(b2), "b3t": bpack(b3),
        "wfm": WF, "whm": WH, "wob": wob.astype(_F32), "bot4": bot4,
    }


_PROG = None
_LAST_RESULTS = None


def _install_ntff_hook():
    """The agent image's antenv lacks axon_hooks; synthesize it and register
    the ctypes-based NTFF profile hook from trn_agent_boot."""
    import types
    import antenv
    if getattr(antenv, "axon_hooks", None) is not None:
        return
    m = types.ModuleType("antenv.axon_hooks")
    state = {"h": None}
    m.set_axon_ntff_profile_hook = lambda h: state.__setitem__("h", h)
    m.get_axon_ntff_profile_hook = lambda: state["h"]
    sys.modules["antenv.axon_hooks"] = m
    antenv.axon_hooks = m
    try:
        from trn_agent_boot.trn_boot import _ntff_profile_via_ctypes
        m.set_axon_ntff_profile_hook(_ntff_profile_via_ctypes("/opt/axon/libaxon_pjrt.so"))
    except Exception as e:
        print("ntff hook install failed:", e)


def kernel(**inputs):
    global _PROG
    inp = {k: np.asarray(v, dtype=np.asarray(v).dtype) for k, v in inputs.items()}
    x = np.asarray(inp["x"], _F32)
    w2 = np.asarray(inp["w2"], _F32)
    w3 = np.asarray(inp["w3"], _F32)
    bn1 = tuple(np.asarray(inp[k], _F32) for k in ("bn1_g", "bn1_b", "bn1_m", "bn1_v"))
    bn2 = tuple(np.asarray(inp[k], _F32) for k in ("bn2_g", "bn2_b", "bn2_m", "bn2_v"))

    in_maps = []
    for c in range(_NCORES):
        xc = x[c * _BPC:(c + 1) * _BPC]
        in_maps.append(_prep_core_inputs(
            xc, np.asarray(inp["w1"], _F32), np.asarray(inp["b1"], _F32),
            w2, np.asarray(inp["b2"], _F32), w3, np.asarray(inp["b3"], _F32),
            bn1,
            np.asarray(inp["wf"], _F32), np.asarray(inp["bf"], _F32),
            np.asarray(inp["wi1"], _F32), np.asarray(inp["bi1"], _F32),
            np.asarray(inp["wi2"], _F32), np.asarray(inp["bi2"], _F32),
            np.asarray(inp["wo"], _F32), np.asarray(inp["bo"], _F32),
            bn2, np.asarray(inp["w_out"], _F32), np.asarray(inp["b_out"], _F32),
        ))

    if _PROG is None:
        _PROG = _build_program()
    from concourse.bass_utils import run_bass_kernel_spmd
    import os as _os
    if _os.environ.get("TRN_KERNEL_TRACE"):
        _install_ntff_hook()
    res = run_bass_kernel_spmd(_PROG, in_maps, core_ids=list(range(_NCORES)),
                               trace=bool(_os.environ.get("TRN_KERNEL_TRACE")))
    global _LAST_RESULTS
    _LAST_RESULTS = res

    out = np.empty((_B, _T, 1), _F32)
    for c in range(_NCORES):
        yc = np.asarray(res.results[c]["out"])  # [4 batch, 32 t]
        out[c * _BPC:(c + 1) * _BPC, :, 0] = yc
    return out


if __name__ == "__main__":
    pass
